# revision 34
# baseline (speedup 1.0000x reference)
"""GQA attention (B=2, S=1024, D=2048, 32 q heads / 8 kv heads, RoPE, causal)
on 8 TRN2 NeuronCores.

Strategy: pure data parallel - core c handles batch b = c // 4 and two
128-token blocks {j, 7-j} (j = c % 4) of that batch, which balances causal
attention work exactly. Each core computes full K/V for its batch
(replicated within the 4-core batch group), Q for its 256 tokens,
attention, and its 256 rows of the output projection. No collectives.

v2 changes over the first working version:
  - wq / wo are pre-tiled on the host into the exact SBUF layout the
    matmuls consume, so every weight DMA is a large contiguous transfer
    (the v1 rearrange DMAs shredded 16 MB into 256-byte packets).
  - reciprocal -> reciprocal_approx_fast (5x faster on DVE; softmax
    denominators are well-conditioned positive sums).
  - attention tail writes attT halves directly from DVE (partition-shifted
    tensor_mul) - no pack matmuls, no extra psum->sbuf cast.
  - V tiles are copied psum->sbuf once per kv-position tile (strided 3D
    copy) instead of 8 per-head copies.
  - per-phase PSUM pools sized to the full 8 banks for deeper pipelining.
  - DMA emission order: projection operand chunks first, tables/masks last.
"""

import numpy as np
import ml_dtypes

import concourse.bass as bass
import concourse.tile as tile
from concourse import bacc
from concourse import mybir
from concourse.bass_utils import run_bass_kernel_spmd

BF16 = ml_dtypes.bfloat16
D_MODEL = 2048
N_HEAD = 32
N_KV = 8
N_REP = 4
DK = 64
HALF = 32
THETA = 10000.0
B, S = 2, 1024
NT = S // 128  # 8 kv tiles of 128
QT = 256  # q tokens per core (two strided tiles of 128)
# causal chunk list: q-tile A (rows j..508+j step 4, kv extent 4 tiles) and
# q-tile B (rows 512+j.., kv extent 8). Uniform across cores by construction.
CHUNKS = [(i, 0) for i in range(4)] + [(i, 1) for i in range(8)]
NCH = len(CHUNKS)  # 12
# probs column layout: kv tiles 0-3 full-width [A_e|A_o|B_e|B_o] (512 cols
# each), then kv tiles 4-7 B-only [B_e|B_o] (256 cols each) = 3072 cols
def _pcol(i, qt):
    # start column of the [qt even|odd] 256-col block for kv tile i
    return i * 512 + qt * 256 if i < 4 else 2048 + (i - 4) * 256

_cache = {}


def _build_nc(phases=3):
    nc = bacc.Bacc("TRN2", target_bir_lowering=False, debug=False)
    f32 = mybir.dt.float32
    bf16 = mybir.dt.bfloat16

    # ---- DRAM parameters (per-core shards supplied via in_maps) ----
    # all operands pre-tiled on host to [128, kk-major] so DMAs are few,
    # large, and contiguous: t[p, kk*W + c] = orig[kk*128 + p, c]
    xTt = nc.declare_dram_parameter("xTt", [128, 16 * S], bf16, isOutput=False)
    xqt = nc.declare_dram_parameter("xqt", [128, 16 * QT], bf16, isOutput=False)
    # pre-tiled wq: wqt[p, m*2048 + kk*128 + c] = wq_perm[kk*128+p, m*128+c]
    wqt = nc.declare_dram_parameter("wqt", [128, 16 * 16 * 128], bf16, isOutput=False)
    wkt = nc.declare_dram_parameter("wkt", [128, 16 * 512], bf16, isOutput=False)
    wvt = nc.declare_dram_parameter("wvt", [128, 16 * 512], bf16, isOutput=False)
    # pre-tiled wo: wot[q, n*8192 + p*512 + c] = wo[p*128+q, n*512+c]
    wot = nc.declare_dram_parameter("wot", [128, 4 * 16 * 512], bf16, isOutput=False)
    bqr = nc.declare_dram_parameter("bqr", [1, D_MODEL], bf16, isOutput=False)
    bkr = nc.declare_dram_parameter("bkr", [1, 512], bf16, isOutput=False)
    bvr = nc.declare_dram_parameter("bvr", [1, 512], bf16, isOutput=False)
    bor = nc.declare_dram_parameter("bor", [1, D_MODEL], bf16, isOutput=False)
    ckt = nc.declare_dram_parameter("ckt", [128, S], bf16, isOutput=False)
    dkt = nc.declare_dram_parameter("dkt", [128, S], bf16, isOutput=False)
    cqt = nc.declare_dram_parameter("cqt", [128, QT], bf16, isOutput=False)
    dqt = nc.declare_dram_parameter("dqt", [128, QT], bf16, isOutput=False)
    pswap = nc.declare_dram_parameter("pswap", [128, 128], bf16, isOutput=False)
    # selp[:, i*64:(i+1)*64] is a [128, 64] matrix whose row 32*i is all-ones:
    # broadcast matmul source for the batched softmax denominators
    selp = nc.declare_dram_parameter("selp", [128, 4 * 64], f32, isOutput=False)
    # mask[kt_local, c*256 + sub*128 + q_local] in {0, 1} per causal chunk c
    maskT = nc.declare_dram_parameter("maskT", [128, NCH * QT], bf16, isOutput=False)
    out = nc.declare_dram_parameter("out", [QT, D_MODEL], f32, isOutput=True)

    with tile.TileContext(nc) as tc:
        import contextlib

        with contextlib.ExitStack() as es:
            singles = es.enter_context(tc.tile_pool(name="singles", bufs=1))
            work = es.enter_context(tc.tile_pool(name="work", bufs=2))

            # ---- persistent constants / tables ----
            ck_sb = singles.tile([128, S], bf16)
            dk_sb = singles.tile([128, S], bf16)
            cq_sb = singles.tile([128, QT], bf16)
            dq_sb = singles.tile([128, QT], bf16)
            psw_sb = singles.tile([128, 128], bf16)
            sel_sb = singles.tile([128, 4 * 64], f32)
            mask_sb = singles.tile([128, NCH * QT], bf16)
            bq_sb = singles.tile([1, D_MODEL], bf16)
            bk_sb = singles.tile([1, 512], bf16)
            bv_sb = singles.tile([1, 512], bf16)
            bo_sb = singles.tile([1, D_MODEL], bf16)
            ones_row = singles.tile([1, 512], bf16)
            nc.vector.memset(ones_row, 1.0)

            # ---- persistent activations ----
            ropek = [singles.tile([64, S], bf16, name=f"ropek{i}", tag=f"ropek{i}") for i in range(N_KV)]
            # per-pair Q layout [64, 512]: [qtA even | qtA odd | qtB even | qtB odd]
            ropeqp = [singles.tile([64, 2 * QT], bf16, name=f"ropeqp{i}", tag=f"ropeqp{i}") for i in range(N_HEAD // 2)]
            # vpi[i]: [128 kv-local, kvh, 65] with col 64 = ones (softmax denom)
            vpi = [singles.tile([128, N_KV, 65], bf16, name=f"vpi{i}", tag=f"vpi{i}") for i in range(NT)]
            attT = [singles.tile([128, QT], bf16, name=f"attT{i}", tag=f"attT{i}") for i in range(N_HEAD // 2)]
            # wo double-buffer as persistent tiles so its DMAs can stream
            # during attention (a scoped pool would WAR-wait on attention tiles)
            wo_sb = [singles.tile([128, 8192], bf16, name=f"wo{i}", tag=f"wo{i}") for i in range(2)]
            # denominator collection tile (rows 0/32/64/96 per 4-pair batch);
            # unused rows hold 1.0 so the batched reciprocal stays finite
            dn = singles.tile([128, 2 * QT], bf16, name="dn", tag="dn")
            nc.vector.memset(dn, 1.0)

            # =========== Phase 1: projections + rope ===========
            with contextlib.ExitStack() as proj_es:
                ppool = proj_es.enter_context(tc.tile_pool(name="proj", bufs=1))
                wqpool = proj_es.enter_context(tc.tile_pool(name="wqp", bufs=3))
                psA = proj_es.enter_context(tc.tile_pool(name="psA", bufs=2, space="PSUM"))
                psB = proj_es.enter_context(tc.tile_pool(name="psB", bufs=2, space="PSUM"))

                xT_sb = ppool.tile([128, 16 * S], bf16, name="xts", tag="xts")
                xq_sb = ppool.tile([128, 16 * QT], bf16, name="xqs", tag="xqs")
                wk_sb = ppool.tile([128, 16 * 512], bf16, name="wks", tag="wks")
                wv_sb = ppool.tile([128, 16 * 512], bf16, name="wvs", tag="wvs")
                # Q-projection gate first (xq + its tables): lets PE start
                # within a few us while the bulk weights stream behind it
                for q4 in range(4):
                    cs = slice(q4 * 4 * QT, (q4 + 1) * 4 * QT)
                    nc.sync.dma_start(out=xq_sb[:, cs], in_=xqt[:, cs])
                for t, src in [
                    (bq_sb, bqr), (psw_sb, pswap), (cq_sb, cqt), (dq_sb, dqt),
                    (bk_sb, bkr), (bv_sb, bvr), (bo_sb, bor),
                    (ck_sb, ckt), (dk_sb, dkt),
                ]:
                    nc.sync.dma_start(out=t, in_=src[:])

                # ---- Q^T = wq^T @ xqT, rope -> ropeq[64, QT] per head ----
                for m in range(16):
                    wqm = wqpool.tile([128, 2048], bf16, tag="wqm", bufs=4)
                    # two half-DMAs -> two queues, keeps the wq stream at PE pace
                    nc.sync.dma_start(out=wqm[:, 0:1024],
                                      in_=wqt[:, m * 2048:m * 2048 + 1024])
                    nc.sync.dma_start(out=wqm[:, 1024:2048],
                                      in_=wqt[:, m * 2048 + 1024:(m + 1) * 2048])
                    # K-projection operands prefetch in the background,
                    # staggered so the first wqm chunks aren't starved behind
                    # 1.5 MB of prefetch on shared DMA queues
                    if m % 4 == 2:
                        q4 = m // 4
                        cs = slice(q4 * 4 * S, (q4 + 1) * 4 * S)
                        nc.sync.dma_start(out=xT_sb[:, cs], in_=xTt[:, cs])
                    if m % 8 == 3:
                        h8 = m // 8
                        cs = slice(h8 * 8 * 512, (h8 + 1) * 8 * 512)
                        nc.sync.dma_start(out=wk_sb[:, cs], in_=wkt[:, cs])
                    qps = psB.tile([128, QT], mybir.dt.float32, tag="Q")
                    for kk in range(16):
                        nc.tensor.matmul(
                            qps,
                            wqm[:, kk * 128:kk * 128 + 128],
                            xq_sb[:, kk * QT:(kk + 1) * QT],
                            start=(kk == 0), stop=False,
                        )
                    nc.tensor.matmul(
                        qps,
                        bq_sb[:, m * 128:m * 128 + 128],
                        ones_row[:, 0:QT],
                        start=False, stop=True,
                    )
                    q_sb = work.tile([128, QT], bf16, tag="qsb")
                    nc.vector.tensor_copy(q_sb, qps)
                    swq = psB.tile([128, QT], mybir.dt.float32, tag="Q")
                    nc.tensor.matmul(swq, psw_sb, q_sb, start=True, stop=True)
                    t1 = work.tile([128, QT], bf16, tag="qt1")
                    t2 = work.tile([128, QT], bf16, tag="qt2")
                    nc.vector.tensor_mul(t1, q_sb, cq_sb)
                    nc.vector.tensor_mul(t2, swq, dq_sb)
                    # pair layout: [A even | A odd | B even | B odd]
                    nc.vector.tensor_add(ropeqp[m][:, 0:128], t1[0:64, 0:128], t2[0:64, 0:128])
                    nc.vector.tensor_add(ropeqp[m][:, 128:256], t1[64:128, 0:128], t2[64:128, 0:128])
                    nc.vector.tensor_add(ropeqp[m][:, 256:384], t1[0:64, 128:256], t2[0:64, 128:256])
                    nc.vector.tensor_add(ropeqp[m][:, 384:512], t1[64:128, 128:256], t2[64:128, 128:256])

                # ---- K^T = wk^T @ xT, rope -> ropek[64, S] per kv head ----
                for m in range(4):
                    kps = psA.tile([128, S], mybir.dt.float32, tag="A")
                    for hf in range(2):
                        cols = slice(hf * 512, hf * 512 + 512)
                        for kk in range(16):
                            nc.tensor.matmul(
                                kps[:, cols],
                                wk_sb[:, kk * 512 + m * 128:kk * 512 + m * 128 + 128],
                                xT_sb[:, kk * S + hf * 512:kk * S + hf * 512 + 512],
                                start=(kk == 0), stop=False,
                            )
                        nc.tensor.matmul(
                            kps[:, cols],
                            bk_sb[:, m * 128:m * 128 + 128],
                            ones_row[:, 0:512],
                            start=False, stop=True,
                        )
                    k_sb = work.tile([128, S], bf16, tag="ksb")
                    nc.vector.tensor_copy(k_sb, kps)
                    swp = psA.tile([128, S], mybir.dt.float32, tag="A")
                    for hf in range(2):
                        cols = slice(hf * 512, hf * 512 + 512)
                        nc.tensor.matmul(swp[:, cols], psw_sb, k_sb[:, cols],
                                         start=True, stop=True)
                    t1 = work.tile([128, S], bf16, tag="t1")
                    t2 = work.tile([128, S], bf16, tag="t2")
                    nc.vector.tensor_mul(t1, k_sb, ck_sb)
                    nc.vector.tensor_mul(t2, swp, dk_sb)
                    nc.vector.tensor_add(ropek[2 * m], t1[0:64, :], t2[0:64, :])
                    nc.vector.tensor_add(ropek[2 * m + 1], t1[64:128, :], t2[64:128, :])

                # ---- V[t, dv] + bias -> vpi tiles with ones column ----
                for h2 in range(2):
                    cs = slice(h2 * 8 * 512, (h2 + 1) * 8 * 512)
                    nc.sync.dma_start(out=wv_sb[:, cs], in_=wvt[:, cs])
                nc.sync.dma_start(out=mask_sb, in_=maskT[:])
                nc.sync.dma_start(out=sel_sb, in_=selp[:])
                for i in range(NT):
                    nc.vector.memset(vpi[i][:, :, 64:65], 1.0)
                for i in range(NT):
                    vps = psB.tile([128, N_KV, 64], mybir.dt.float32, tag="B")
                    for kk in range(16):
                        nc.tensor.matmul(
                            vps,
                            xT_sb[:, kk * S + i * 128:kk * S + i * 128 + 128],
                            wv_sb[:, kk * 512:(kk + 1) * 512],
                            start=(kk == 0), stop=False,
                        )
                    nc.tensor.matmul(
                        vps,
                        ones_row[:, 0:128],
                        bv_sb,
                        start=False, stop=True,
                    )
                    nc.vector.tensor_copy(vpi[i][:, :, 0:64], vps)

                # prefetch wo for output projection (n=0,1) during attention
                for n in range(2):
                    for q4 in range(4):
                        cs = slice(q4 * 2048, (q4 + 1) * 2048)
                        nc.sync.dma_start(
                            out=wo_sb[n][:, cs],
                            in_=wot[:, n * 8192 + q4 * 2048:n * 8192 + (q4 + 1) * 2048])

            # =========== Phase 2: attention ===========
            if phases < 2:
                return nc
            with contextlib.ExitStack() as att_es:
                apool = att_es.enter_context(tc.tile_pool(name="att", bufs=4))
                dpool = att_es.enter_context(tc.tile_pool(name="div", bufs=2))
                attvp = att_es.enter_context(tc.tile_pool(name="attv", bufs=10))
                psc = att_es.enter_context(tc.tile_pool(name="psc", bufs=3, space="PSUM"))
                pov = att_es.enter_context(tc.tile_pool(name="pov", bufs=1, space="PSUM"))
                pbc = att_es.enter_context(tc.tile_pool(name="pbc", bufs=1, space="PSUM"))

                attv = [None] * (N_HEAD // 2)
                for p2 in range(N_HEAD // 2):
                    kvh = p2 // 2
                    g, idx = p2 // 4, p2 % 4
                    probs = apool.tile([128, NCH * QT], bf16, tag="probs")
                    # kv tiles 0-3: one full-width N=512 matmul covers both
                    # q-tiles; kv tiles 4-7: N=256 for q-tile B only
                    for t3 in range(3):
                        sps = psc.tile([128, 4 * QT], mybir.dt.float32, tag="SC")
                        if t3 < 2:
                            for ii in range(2):
                                i = t3 * 2 + ii
                                nc.tensor.matmul(
                                    sps[:, ii * 512:(ii + 1) * 512],
                                    ropek[kvh][:, i * 128:i * 128 + 128],
                                    ropeqp[p2],
                                    start=True, stop=True,
                                )
                        else:
                            for ii in range(4):
                                i = 4 + ii
                                nc.tensor.matmul(
                                    sps[:, ii * QT:(ii + 1) * QT],
                                    ropek[kvh][:, i * 128:i * 128 + 128],
                                    ropeqp[p2][:, QT:2 * QT],
                                    start=True, stop=True,
                                )
                        nc.scalar.activation(
                            probs[:, t3 * 4 * QT:(t3 + 1) * 4 * QT],
                            sps,
                            mybir.ActivationFunctionType.Exp,
                            bias=0.0, scale=0.125,
                        )
                    # only these regions can be causally masked: the A-halves
                    # of kv tiles 0-3 and all of kv tiles 4-7 (q-tile B rows
                    # >= 512 are strictly after every kv position < 512)
                    for i4 in range(4):
                        cs = slice(i4 * 512, i4 * 512 + 256)
                        nc.vector.tensor_mul(probs[:, cs], probs[:, cs], mask_sb[:, cs])
                    cs = slice(2048, 3072)
                    nc.vector.tensor_mul(probs[:, cs], probs[:, cs], mask_sb[:, cs])
                    # outv pair layout [65, 512]: [A even | A odd | B even | B odd]
                    outv = pov.tile([65, 2 * QT], mybir.dt.float32, tag="OV")
                    for i in range(4):
                        nc.tensor.matmul(
                            outv,
                            vpi[i][:, kvh, :],
                            probs[:, i * 512:(i + 1) * 512],
                            start=(i == 0), stop=False,
                            skip_group_check=True,
                        )
                    for i in range(4, NT):
                        nc.tensor.matmul(
                            outv[:, QT:2 * QT],
                            vpi[i][:, kvh, :],
                            probs[:, 2048 + (i - 4) * QT:2048 + (i - 3) * QT],
                            start=False, stop=(i == NT - 1),
                            skip_group_check=True,
                        )
                    # stash unnormalized output (row 64 = denominators) and
                    # collect this pair's denominators into the batch tile
                    attv[p2] = attvp.tile([65, 2 * QT], bf16, tag="attv", name=f"attv{p2}")
                    nc.vector.tensor_copy(attv[p2], outv)
                    nc.vector.tensor_copy(dn[32 * idx:32 * idx + 1, :], outv[64:65, :])
                    if idx == 3:
                        # batched softmax denominators: one reciprocal for 4
                        # pairs (rows 0/32/64/96), then PE broadcast + normalize
                        rdn = dpool.tile([128, 2 * QT], mybir.dt.float32, tag="rdn")
                        # split so the 3.3us reciprocal doesn't block the DVE
                        # FIFO (next pair's mask-mul can interleave between)
                        for rq in range(4):
                            cs = slice(rq * 128, (rq + 1) * 128)
                            nc.vector.reciprocal(rdn[:, cs], dn[:, cs])
                        for i2 in range(4):
                            q2 = g * 4 + i2
                            bcp = pbc.tile([64, 2 * QT], mybir.dt.float32, tag="BC")
                            nc.tensor.matmul(
                                bcp, sel_sb[:, i2 * 64:(i2 + 1) * 64], rdn,
                                start=True, stop=True,
                            )
                            # attT[q2] cols = [qtA | qtB]; rows 0:64 even head,
                            # 64:128 odd head
                            nc.vector.tensor_mul(
                                attT[q2][0:64, 0:128], attv[q2][0:64, 0:128],
                                bcp[:, 0:128])
                            nc.vector.tensor_mul(
                                attT[q2][0:64, 128:256], attv[q2][0:64, 256:384],
                                bcp[:, 256:384])
                            nc.vector.tensor_mul(
                                attT[q2][64:128, 0:128], attv[q2][0:64, 128:256],
                                bcp[:, 128:256])
                            nc.vector.tensor_mul(
                                attT[q2][64:128, 128:256], attv[q2][0:64, 384:512],
                                bcp[:, 384:512])

            # =========== Phase 3: output projection ===========
            if phases < 3:
                return nc
            with contextlib.ExitStack() as op_es:
                opool = op_es.enter_context(tc.tile_pool(name="osb", bufs=3))
                pop = op_es.enter_context(tc.tile_pool(name="pop", bufs=4, space="PSUM"))
                # (wo_sb[0], wo_sb[1] were DMA'd during attention)
                for n in range(4):
                    won = wo_sb[n % 2]
                    if n >= 2:
                        # refill the double-buffer (WAR on n-2's consumption)
                        for q4 in range(4):
                            cs = slice(q4 * 2048, (q4 + 1) * 2048)
                            nc.sync.dma_start(
                                out=won[:, cs],
                                in_=wot[:, n * 8192 + q4 * 2048:n * 8192 + (q4 + 1) * 2048])
                    for blk in range(2):
                        ops = pop.tile([128, 512], mybir.dt.float32, tag="OP")
                        for p in range(16):
                            nc.tensor.matmul(
                                ops,
                                attT[p][:, blk * 128:blk * 128 + 128],
                                won[:, p * 512:(p + 1) * 512],
                                start=(p == 0), stop=False,
                            )
                        nc.tensor.matmul(
                            ops,
                            ones_row[:, 0:128],
                            bo_sb[:, n * 512:n * 512 + 512],
                            start=False, stop=True,
                        )
                        osb = opool.tile([128, 512], mybir.dt.float32, tag="osb")
                        nc.vector.tensor_copy(osb, ops)
                        nc.sync.dma_start(
                            out=out[blk * 128:blk * 128 + 128, n * 512:n * 512 + 512],
                            in_=osb,
                        )
    return nc


def _host_prep(x, Wq, bq, Wk, bk, Wv, bv, Wo, bo):
    """Build per-core input maps."""
    # per-head even/odd deinterleave permutation of output columns
    def colperm(nheads):
        p = []
        for h in range(nheads):
            base = h * DK
            p.extend([base + 2 * j for j in range(HALF)])
            p.extend([base + 2 * j + 1 for j in range(HALF)])
        return np.array(p)

    def ktile(a):
        # [2048, W] -> [128, 16*W] with t[p, kk*W + c] = a[kk*128 + p, c]
        w = a.shape[1]
        return np.ascontiguousarray(
            a.reshape(16, 128, w).transpose(1, 0, 2).reshape(128, 16 * w))

    qperm = colperm(N_HEAD)
    kperm = colperm(N_KV)
    wq_p = np.ascontiguousarray(Wq[:, qperm]).astype(BF16)
    wk_p = np.ascontiguousarray(Wk[:, kperm]).astype(BF16)
    bq_p = np.ascontiguousarray(bq[qperm]).astype(BF16).reshape(1, D_MODEL)
    bk_p = np.ascontiguousarray(bk[kperm]).astype(BF16).reshape(1, 512)
    wv_c = Wv.astype(BF16)
    wo_c = Wo.astype(BF16)
    bv_r = bv.astype(BF16).reshape(1, 512)
    bo_r = bo.astype(BF16).reshape(1, D_MODEL)

    # pre-tiled weight layouts (contiguous per-partition DMA lines)
    # wqt[p, m*2048 + kk*128 + c] = wq_p[kk*128 + p, m*128 + c]
    wqt = np.ascontiguousarray(
        wq_p.reshape(16, 128, 16, 128).transpose(1, 2, 0, 3).reshape(128, 16 * 16 * 128)
    )
    # wot[q, n*8192 + p*512 + c] = wo[p*128 + q, n*512 + c]
    wot = np.ascontiguousarray(
        wo_c.reshape(16, 128, 4, 512).transpose(1, 2, 0, 3).reshape(128, 4 * 16 * 512)
    )

    invf = THETA ** (-(np.arange(HALF, dtype=np.float64) * 2.0 / DK))
    posf = np.arange(S, dtype=np.float64)
    ang = posf[:, None] * invf[None, :]  # [S, 32]
    cos_t, sin_t = np.cos(ang), np.sin(ang)

    def rope_tables(pos_idx):
        # [128, len(pos_idx)] tables in deinterleaved space (2 heads / 128 rows)
        n = len(pos_idx)
        C = np.zeros((128, n), np.float32)
        D = np.zeros((128, n), np.float32)
        for p in range(128):
            r = p % DK
            i = r if r < HALF else r - HALF
            C[p] = cos_t[pos_idx, i]
            D[p] = (-sin_t[pos_idx, i]) if r < HALF else sin_t[pos_idx, i]
        return C.astype(BF16), D.astype(BF16)

    ckt, dkt = rope_tables(np.arange(S))

    psw = np.zeros((128, 128), np.float32)
    for m in range(128):
        k = m + HALF if (m % DK) < HALF else m - HALF
        psw[k, m] = 1.0
    psw = psw.astype(BF16)
    sel = np.zeros((128, 4 * 64), np.float32)
    for idx in range(4):
        sel[32 * idx, idx * 64:(idx + 1) * 64] = 1.0

    in_maps = []
    meta = []
    for c in range(8):
        b, j = c // 4, c % 4
        # striped q-row assignment: q-tile A = rows j..508+j step 4 (causal
        # extent 4 kv tiles), q-tile B = rows 512+j.. step 4 (extent 8) —
        # identical extents on every core, so the SPMD chunk structure is
        # uniform while skipping the causally-dead upper region.
        qrows = np.concatenate([np.arange(j, 512, 4), np.arange(512 + j, 1024, 4)])
        xb = np.asarray(x[b], dtype=np.float32)
        xT = ktile(np.ascontiguousarray(xb.T).astype(BF16))
        xqT = ktile(np.ascontiguousarray(xb[qrows].T).astype(BF16))
        cqt, dqt = rope_tables(qrows)
        # mask laid out to match the probs columns (_pcol), duplicated for the
        # even/odd head halves of each 256-col block
        mask = np.zeros((128, NCH * QT), np.float32)
        kt_local = np.arange(128)
        for i in range(NT):
            ktg = i * 128 + kt_local
            for qt in ([0, 1] if i < 4 else [1]):
                col = _pcol(i, qt)
                qpos = qrows[qt * 128:(qt + 1) * 128]
                m01 = (ktg[:, None] <= qpos[None, :])
                mask[:, col:col + 128] = m01
                mask[:, col + 128:col + 256] = m01
        in_maps.append({
            "xTt": xT, "xqt": xqT, "wqt": wqt, "wkt": ktile(wk_p),
            "wvt": ktile(wv_c), "wot": wot,
            "bqr": bq_p, "bkr": bk_p, "bvr": bv_r, "bor": bo_r,
            "ckt": ckt, "dkt": dkt, "cqt": cqt, "dqt": dqt,
            "pswap": psw, "selp": sel,
            "maskT": mask.astype(BF16),
        })
        meta.append((b, qrows))
    return in_maps, meta


def kernel(x, Wq, bq, Wk, bk, Wv, bv, Wo, bo):
    if "nc" not in _cache:
        nc0 = _build_nc()
        nc0.finalize()
        _cache["nc"] = nc0
    nc = _cache["nc"]
    in_maps, meta = _host_prep(x, Wq, bq, Wk, bk, Wv, bv, Wo, bo)
    res = run_bass_kernel_spmd(nc, in_maps, list(range(8)))
    full = np.zeros((B, S, D_MODEL), np.float32)
    for c in range(8):
        b, qrows = meta[c]
        o = res.results[c]["out"]
        full[b, qrows] = o
    return full
